# revision 11
# baseline (speedup 1.0000x reference)
"""Trainium2 Bass kernel for nn_DebiasIntraDist (segment_reduce).

Full-input contract: kernel(**inputs) takes the complete (unsharded) inputs
and returns the full scalar loss. Sharding: core 2d+h gets the rows with
demog == d and label-half h, so every core owns a disjoint set of 256
(demog, label) groups. Within a core, rows are partitioned into two
segments by label-quarter (local label < 128 vs >= 128), each padded to a
whole number of 128-row tiles (T0, T1 known at compile time). Every tile
therefore feeds exactly ONE 128-group PSUM accumulator.

The host ships feats pre-split into an error-compensated bf16 pair per
row: hi = rne_bf16(x), lo = bf16(x - hi) - same 4 bytes/element as fp32,
so DMA volume is unchanged, but the device never has to do the fp32
hi/lo subtract and every matmul operand is contiguous bf16. Per tile:
    oh   = one_hot(labels_local)   # vector IS_EQ
    xadd = hi + lo (~= x in bf16)  # vector, 16-bit 2x rate
    xsq  = xadd * xadd             # scalar engine Square, bf16
    sums[seg]  += oh^T @ hi + oh^T @ lo     # tensor, one PSUM bank per seg
    sumsq[seg] += oh^T @ xsq                # tensor
All engines sit below the ~715 ns/tile DMA roof, so the loop is paced by
the HBM stream itself. After each segment: norm2[g] = sum_d sums[g,d]^2
(scalar Square+accum from PSUM) and sumsq_g[g] = sum_d sumsq[g,d]
(vector reduce) -> out tile [128, 4] -> DMA out.

Each core returns its [128, 4] partial stats; the host (which knows the
per-group counts from the shard step) finishes the tiny O(G) reduction
to the scalar loss in fp64. No collectives anywhere.
"""

import numpy as np
import ml_dtypes

try:
    import concourse.bacc as bacc
except ImportError:  # fresh environment without PYTHONPATH set up
    import sys
    for p in ("/root/.axon_site/_ro/trn_rl_repo", "/opt/trn_rl_repo",
              "/root/.axon_site/_ro/pypackages"):
        if p not in sys.path:
            sys.path.append(p)
    import concourse.bacc as bacc
import concourse.mybir as mybir
import concourse.tile as tile
import concourse.bass_utils as bass_utils

N_CORES = 8
P = 128
D = 512          # feature dim
ND = 4           # demog values
PAD_LABEL = 500.0  # never matches iota [0,128)

_cache: dict[tuple, object] = {}


def _build(T0: int, T1: int):
    """Compile the SPMD kernel: T0/T1 = tiles in segment 0/1."""
    T = T0 + T1
    fp32 = mybir.dt.float32
    bf16 = mybir.dt.bfloat16
    Alu = mybir.AluOpType
    Act = mybir.ActivationFunctionType

    nc = bacc.Bacc("TRN2", target_bir_lowering=False, debug=False,
                   enable_asserts=True, num_devices=N_CORES)

    feats = nc.dram_tensor("feats", [T * P, 2 * D], bf16,
                           kind="ExternalInput").ap()
    labels_t = nc.dram_tensor("labels_t", [P, T], fp32,
                              kind="ExternalInput").ap()
    iota_t = nc.dram_tensor("iota_t", [P, P], bf16,
                            kind="ExternalInput").ap()
    part = nc.dram_tensor("part", [P, 4], fp32, kind="ExternalOutput").ap()

    with tile.TileContext(nc) as tc:
        with (
            tc.tile_pool(name="const", bufs=1) as constp,
            tc.tile_pool(name="fx", bufs=10) as fxp,
            tc.tile_pool(name="oh", bufs=8) as ohp,
            tc.tile_pool(name="ad", bufs=6) as adp,
            tc.tile_pool(name="sq", bufs=6) as sqp,
            tc.tile_pool(name="post", bufs=1) as postp,
            tc.tile_pool(name="ps", bufs=1, space="PSUM") as psp,
        ):
            # constants via DMA only - ready as soon as the queues go live
            labs = constp.tile([P, T], fp32, tag="labs")
            nc.scalar.dma_start(out=labs[:], in_=labels_t[:])
            iota = constp.tile([P, P], bf16, tag="iota")
            nc.scalar.dma_start(out=iota[:], in_=iota_t[:])

            # warm both activation tables (bf16 loop squares, fp32 post
            # squares) before the first real use
            warm = constp.tile([P, 1], bf16, tag="warm")
            nc.vector.memset(warm[:], 0.0)
            nc.scalar.activation(warm[:], warm[:], Act.Square)
            warm2 = constp.tile([P, 1], fp32, tag="warm2")
            nc.vector.memset(warm2[:], 0.0)
            nc.scalar.activation(warm2[:], warm2[:], Act.Square)

            # per-segment accumulators: one PSUM bank each
            ps_sums = [psp.tile([P, D], fp32, tag=f"sums{s}", name=f"sums{s}")
                       for s in range(2)]
            ps_sq = [psp.tile([P, D], fp32, tag=f"sq{s}", name=f"sq{s}")
                     for s in range(2)]
            out4 = postp.tile([P, 4], fp32, tag="out4")

            feats_r = feats.rearrange("(n p) d -> n p d", p=P)  # [T, P, 2D]

            def seg_post(seg):
                # norm2[g] = sum_d sums[g,d]^2 ; sumsq_g[g] = sum_d sumsq[g,d]
                scr = postp.tile([P, D], fp32, tag=f"scr{seg}",
                                 name=f"scr{seg}")
                nc.scalar.activation(
                    scr[:], ps_sums[seg][:], Act.Square,
                    accum_out=out4[:, 2 * seg + 1:2 * seg + 2])
                nc.vector.tensor_reduce(
                    out=out4[:, 2 * seg:2 * seg + 1], in_=ps_sq[seg][:],
                    axis=mybir.AxisListType.X, op=Alu.add)

            for ti in range(T):
                seg = 0 if ti < T0 else 1
                st = ti in (0, T0)
                sp = ti in (T0 - 1, T - 1)
                fx = fxp.tile([P, 2 * D], bf16, tag="fx")
                # scalar-queue DMAs start ~2.5us earlier than sync-queue
                # ones; use them for the first tiles to cut the ramp
                eng = nc.scalar if ti < 2 else nc.sync
                eng.dma_start(out=fx[:], in_=feats_r[ti])
                Xhi = fx[:, 0:D]
                Xlo = fx[:, D:2 * D]
                oh = ohp.tile([P, P], bf16, tag="oh")
                nc.vector.tensor_scalar(
                    out=oh[:], in0=iota[:], scalar1=labs[:, ti:ti + 1],
                    scalar2=None, op0=Alu.is_equal,
                )
                xadd = adp.tile([P, D], bf16, tag="xadd")
                nc.vector.tensor_tensor(out=xadd[:], in0=Xhi, in1=Xlo,
                                        op=Alu.add)
                xsq = sqp.tile([P, D], bf16, tag="xsq")
                nc.scalar.activation(xsq[:], xadd[:], Act.Square)
                nc.tensor.matmul(out=ps_sums[seg][:], lhsT=oh[:],
                                 rhs=Xhi, start=st, stop=False)
                nc.tensor.matmul(out=ps_sums[seg][:], lhsT=oh[:],
                                 rhs=Xlo, start=False, stop=sp)
                nc.tensor.matmul(out=ps_sq[seg][:], lhsT=oh[:],
                                 rhs=xsq[:], start=st, stop=sp)
                if sp:
                    seg_post(seg)

            nc.sync.dma_start(out=part[:], in_=out4[:])

    nc.compile()
    return nc


def _prepare(feats, labels, demog):
    """Shard rows by (demog, label-half); sort each shard into two
    label-quarter segments padded to whole tiles. Ships hi|lo bf16 planes.
    Returns the compile key, per-core input maps, and per-(core, seg, slot)
    counts for the host combine.
    """
    lab256 = labels % 256
    core_id = demog * 2 + (labels >= 256).astype(np.int32)
    seg_id = (lab256 >= 128).astype(np.int32)
    labloc = (lab256 % 128).astype(np.float32)

    idx = [[np.flatnonzero((core_id == k) & (seg_id == s)) for s in range(2)]
           for k in range(N_CORES)]
    T0 = max(1, max(-(-len(idx[k][0]) // P) for k in range(N_CORES)))
    T1 = max(1, max(-(-len(idx[k][1]) // P) for k in range(N_CORES)))
    T = T0 + T1
    S = T * P

    # error-compensated bf16 split of the full feats matrix (hi = rne(x),
    # lo = bf16(x - hi)); per-row layout [hi | lo] keeps DMA descriptors
    # contiguous 2 KiB runs, same bytes/row as the original fp32
    hi = feats.astype(ml_dtypes.bfloat16)
    lo = (feats - hi.astype(np.float32)).astype(ml_dtypes.bfloat16)

    iota_np = np.broadcast_to(np.arange(P, dtype=np.float32), (P, P))
    iota_np = np.ascontiguousarray(iota_np).astype(ml_dtypes.bfloat16)
    in_maps = []
    cnts = np.zeros((N_CORES, 2, P), np.int64)
    for k in range(N_CORES):
        f = np.zeros((S, 2 * D), ml_dtypes.bfloat16)
        lab = np.full(S, PAD_LABEL, np.float32)
        for s, base in ((0, 0), (1, T0 * P)):
            rows = idx[k][s]
            f[base:base + len(rows), :D] = hi[rows]
            f[base:base + len(rows), D:] = lo[rows]
            lab[base:base + len(rows)] = labloc[rows]
            cnts[k, s] = np.bincount(labloc[rows].astype(np.int64),
                                     minlength=P)
        labs_t = np.ascontiguousarray(lab.reshape(T, P).T)
        in_maps.append({"feats": f, "labels_t": labs_t, "iota_t": iota_np})
    return (T0, T1), in_maps, cnts


def _combine(parts, cnts):
    """Finish the reduction on host in fp64: parts[k] = [128,4] device out."""
    num = np.zeros(ND, np.float64)
    den = np.zeros(ND, np.float64)
    for k in range(N_CORES):
        d = k // 2
        p = np.asarray(parts[k], np.float64)
        for s in range(2):
            ssg = p[:, 2 * s]          # per-group sum of ||x||^2
            nn2 = p[:, 2 * s + 1]      # per-group ||sums||^2
            c = cnts[k, s].astype(np.float64)
            safe = np.maximum(c, 1.0)
            grp = (ssg - nn2 / safe) / safe
            pres = (c > 0)
            num[d] += grp[pres].sum()
            den[d] += pres.sum()
    intra = num / np.maximum(den, 1.0)
    return np.float32(np.mean(np.abs(intra - intra.mean())))


def kernel(feats, labels, demog_labels, _results_out=None):
    feats = np.ascontiguousarray(np.asarray(feats), dtype=np.float32)
    labels = np.asarray(labels).astype(np.int32)
    demog = np.asarray(demog_labels).astype(np.int32)
    assert feats.ndim == 2 and feats.shape[1] == D

    key, in_maps, cnts = _prepare(feats, labels, demog)
    nc = _cache.get(key)
    if nc is None:
        nc = _cache.setdefault(key, _build(*key))
    res = None
    last_exc = None
    for attempt in range(3):
        try:
            res = bass_utils.run_bass_kernel_spmd(
                nc, in_maps, core_ids=list(range(N_CORES)))
            break
        except Exception as e:  # transient axon worker hangups
            last_exc = e
            import time
            time.sleep(10)
    if res is None:
        raise last_exc
    if _results_out is not None:
        _results_out.append(res)
    return _combine([res.results[k]["part"] for k in range(N_CORES)], cnts)
